# revision 15
# baseline (speedup 1.0000x reference)
"""Trainium2 Bass kernel for a 3-layer tanh RNN (SEQ=512, BATCH=64, IN=128, HID=512).

Strategy (v1): data-parallel over batch — 8 NeuronCores, 8 batch rows each,
weights replicated. All compute on device:
  - per-layer input projection xw = y_prev @ W_ih^T + b   (big GEMMs, folded-T layout)
  - sequential 512-step recurrence h_t = tanh(xw_t + h_{t-1} @ W_hh^T)
Layouts keep h transposed+folded ([128 partitions, 4*BL cols] where row p of
j-block is hidden unit j*128+p) so no transposes are needed inside the scan.
Weights/activations bf16 (fp32 PSUM accumulation), xw and output fp32.
"""

import numpy as np
import ml_dtypes

S, B, IN, H, NL = 512, 64, 128, 512, 3
NCORES = 8
BL = B // NCORES          # batch rows per core
JT = H // 128             # output j-tiles
KC = H // 128             # contraction chunks
FB = JT * BL              # folded slab width per timestep

_NC_CACHE = {}


# ---------------- host-side packing ----------------

def _pack_wT(w):
    """[Hout, Hin] -> [128, (Hin/128)*(Hout/128)*128] bf16; tile (kc, j) at
    cols ((kc*JT + j)*128 : +128), lhsT[k=p, m] = w[j*128+m, kc*128+p]."""
    jt = w.shape[0] // 128
    kcn = w.shape[1] // 128
    w4 = np.asarray(w, np.float32).reshape(jt, 128, kcn, 128)
    return np.ascontiguousarray(
        w4.transpose(3, 2, 0, 1).reshape(128, kcn * jt * 128)
    ).astype(ml_dtypes.bfloat16)


def _pack_bias(b_ih, b_hh):
    """[NL, H] x2 -> [128, NL*JT] fp32, col l*JT+j row p = bias[l, j*128+p]."""
    s = np.asarray(b_ih, np.float32) + np.asarray(b_hh, np.float32)
    return np.ascontiguousarray(s.reshape(NL, JT, 128).transpose(2, 0, 1).reshape(128, NL * JT))


def _pack_xT(x_slice):
    """[S, BL, IN] -> [IN, S*BL] bf16 (t-major, b-minor columns)."""
    return np.ascontiguousarray(
        np.asarray(x_slice, np.float32).transpose(2, 0, 1).reshape(IN, -1)
    ).astype(ml_dtypes.bfloat16)


# ---------------- device program ----------------

def _build(nsteps=S, bl=BL):
    from contextlib import ExitStack
    from concourse import bacc, mybir
    import concourse.tile as tile

    f32, bf16 = mybir.dt.float32, mybir.dt.bfloat16
    Tanh = mybir.ActivationFunctionType.Tanh
    fb = JT * bl
    TBLK = min(512 // bl, nsteps)   # timesteps per GEMM n-block (N = TBLK*bl <= 512)
    NBLK = nsteps // TBLK

    nc = bacc.Bacc("TRN2", target_bir_lowering=False, debug=False, num_devices=NCORES)

    xT_d = nc.dram_tensor("xT", [IN, nsteps * bl], bf16, kind="ExternalInput")
    wih0T_d = nc.dram_tensor("wih0T", [IN, JT * 128], bf16, kind="ExternalInput")
    wihT_d = nc.dram_tensor("wihT", [128, (NL - 1) * KC * JT * 128], bf16, kind="ExternalInput")
    whhT_d = nc.dram_tensor("whhT", [128, NL * KC * JT * 128], bf16, kind="ExternalInput")
    bias_d = nc.dram_tensor("biasT", [128, NL * JT], f32, kind="ExternalInput")
    ident_d = nc.dram_tensor("identT", [128, 128], bf16, kind="ExternalInput")
    y_d = nc.dram_tensor("y_out", [128, nsteps * fb], f32, kind="ExternalOutput")
    YCH = min(64, nsteps)               # steps per output staging chunk

    with tile.TileContext(nc) as tc, ExitStack() as ctx:
        consts = ctx.enter_context(tc.tile_pool(name="consts", bufs=1))
        big = ctx.enter_context(tc.tile_pool(name="big", bufs=1))
        yts = ctx.enter_context(tc.tile_pool(name="yts", bufs=2))
        gpsum = ctx.enter_context(tc.tile_pool(name="gpsum", bufs=4, space="PSUM"))
        rpsum = ctx.enter_context(tc.tile_pool(name="rpsum", bufs=4, space="PSUM"))
        small = ctx.enter_context(tc.tile_pool(name="small", bufs=6))
        ring = ctx.enter_context(tc.tile_pool(name="ring", bufs=4))

        xT_s = consts.tile([IN, nsteps * bl], bf16)
        nc.sync.dma_start(xT_s[:], xT_d.ap())
        wih0T_s = consts.tile([IN, JT * 128], bf16)
        nc.sync.dma_start(wih0T_s[:], wih0T_d.ap())
        wihT_s = consts.tile([128, (NL - 1) * KC * JT * 128], bf16)
        nc.sync.dma_start(wihT_s[:], wihT_d.ap())
        whhT_s = consts.tile([128, NL * KC * JT * 128], bf16)
        nc.sync.dma_start(whhT_s[:], whhT_d.ap())
        bias_s = consts.tile([128, NL * JT], f32)
        nc.sync.dma_start(bias_s[:], bias_d.ap())
        ident_s = consts.tile([128, 128], bf16)
        nc.sync.dma_start(ident_s[:], ident_d.ap())
        zeros_s = consts.tile([128, fb], bf16)
        nc.vector.memset(zeros_s[:], 0.0)

        ystage_pool = ctx.enter_context(tc.tile_pool(name="ystage", bufs=2))

        yprev_r = None
        for l in range(NL):
            top = l == NL - 1
            nkc = 1 if l == 0 else KC

            # ---- input projection: xwT[p, t, j*bl+b] = (y_prev @ W_ih^T + b)[t, b, j*128+p]
            xwT = big.tile([128, nsteps * fb], bf16, tag="xwT")
            xw_r = xwT[:].rearrange("p (t f) -> p t f", f=fb)
            for j in range(JT):
                for nb in range(NBLK):
                    ps = gpsum.tile([128, TBLK * bl], f32)
                    ps3 = ps[:].rearrange("p (t b) -> p t b", b=bl)
                    for kc in range(nkc):
                        if l == 0:
                            lhsT = wih0T_s[:, j * 128:(j + 1) * 128]
                            rhs = xT_s[:, nb * TBLK * bl:(nb + 1) * TBLK * bl]
                            out = ps[:]
                        else:
                            base = ((l - 1) * KC + kc) * JT + j
                            lhsT = wihT_s[:, base * 128:(base + 1) * 128]
                            rhs = yprev_r[:, nb * TBLK:(nb + 1) * TBLK, kc * bl:(kc + 1) * bl]
                            out = ps3
                        nc.tensor.matmul(out, lhsT, rhs, start=(kc == 0), stop=(kc == nkc - 1))
                    nc.vector.tensor_scalar_add(
                        out=xw_r[:, nb * TBLK:(nb + 1) * TBLK, j * bl:(j + 1) * bl],
                        in0=ps3,
                        scalar1=bias_s[:, l * JT + j:l * JT + j + 1],
                    )

            # ---- recurrence
            if not top:
                yT = yts.tile([128, nsteps * fb], bf16, tag="yT")
                y_r = yT[:].rearrange("p (t f) -> p t f", f=fb)
            prev_slab = zeros_s[:]
            ystage = None
            for t in range(nsteps):
                if top and t % YCH == 0:
                    ystage = ystage_pool.tile([128, YCH * fb], f32, tag="ystage")
                ps = rpsum.tile([128, fb], f32)
                if top:
                    out_slab = ystage[:, (t % YCH) * fb:(t % YCH + 1) * fb]
                else:
                    out_slab = y_r[:, t, :]
                for j in range(JT):
                    for i in range(KC):
                        kc = (j + i) % KC   # rotate so block kc is first needed at MM index ~4*kc
                        base = (l * KC + kc) * JT + j
                        nc.tensor.matmul(
                            ps[:, j * bl:(j + 1) * bl],
                            whhT_s[:, base * 128:(base + 1) * 128],
                            prev_slab[:, kc * bl:(kc + 1) * bl],
                            start=(i == 0), stop=False,
                        )
                    nc.tensor.matmul(
                        ps[:, j * bl:(j + 1) * bl],
                        ident_s[:],
                        xw_r[:, t, j * bl:(j + 1) * bl],
                        start=False, stop=True,
                    )
                    if j % 2 == 1:
                        nc.scalar.activation(out_slab[:, (j - 1) * bl:(j + 1) * bl],
                                             ps[:, (j - 1) * bl:(j + 1) * bl], Tanh)
                if top:
                    hT = ring.tile([128, fb], bf16, tag="hT")
                    nc.vector.tensor_copy(hT[:], out_slab)
                    if t % YCH == YCH - 1:
                        c0 = (t // YCH) * YCH * fb
                        nc.sync.dma_start(y_d.ap()[:, c0:c0 + YCH * fb], ystage[:])
                    prev_slab = hT[:]
                else:
                    prev_slab = y_r[:, t, :]
            if not top:
                yprev_r = y_r

    nc.compile()
    return nc


def _unfold_y(arr, nsteps=S, bl=BL):
    """[p, t*fb + j*bl + b] fp32 -> [t, b, j*128+p]."""
    return np.ascontiguousarray(
        arr.reshape(128, nsteps, JT, bl).transpose(1, 3, 2, 0).reshape(nsteps, bl, H)
    )


def _in_maps(input_x, w_ih_first, w_ih_rest, w_hh, b_ih, b_hh, nsteps=S, bl=BL, ncores=NCORES):
    wih0T = _pack_wT(np.asarray(w_ih_first))
    wihT = np.concatenate([_pack_wT(np.asarray(w_ih_rest)[i]) for i in range(NL - 1)], axis=1)
    whhT = np.concatenate([_pack_wT(np.asarray(w_hh)[i]) for i in range(NL)], axis=1)
    biasT = _pack_bias(b_ih, b_hh)
    maps = []
    for c in range(ncores):
        xs = np.asarray(input_x)[:nsteps, c * bl:(c + 1) * bl, :]
        maps.append({
            "xT": _pack_xT(xs),
            "wih0T": wih0T,
            "wihT": wihT,
            "whhT": whhT,
            "biasT": biasT,
            "identT": np.eye(128, dtype=np.float32).astype(ml_dtypes.bfloat16),
        })
    return maps


def kernel(input_x, w_ih_first, w_ih_rest, w_hh, b_ih, b_hh):
    from concourse.bass_utils import run_bass_kernel_spmd

    key = (S, BL)
    if key not in _NC_CACHE:
        _NC_CACHE[key] = _build(S, BL)
    nc = _NC_CACHE[key]

    maps = _in_maps(input_x, w_ih_first, w_ih_rest, w_hh, b_ih, b_hh)
    res = run_bass_kernel_spmd(nc, maps, core_ids=list(range(NCORES)))
    outs = [_unfold_y(res.results[c]["y_out"]) for c in range(NCORES)]
    return np.concatenate(outs, axis=1).astype(np.float32)


# revision 16
# speedup vs baseline: 2.2619x; 2.2619x over previous
"""v2: layer-pipelined wavefront RNN kernel.

2 wavefronts x 3 stage-cores (+2 spares). Core c runs layer stage(c) for its
wavefront's batch half (32 rows). Blocks of TB timesteps flow down the layer
pipeline via paired AllGathers (even edges R0->R1, odd edges R1->R2), double
buffered so collectives overlap compute (hop delay D=2 ticks).

All 8 cores run an IDENTICAL program; roles differ only through input data:
  - wprojT: the layer's input-projection weights applied to the received
    block (zero for stage 0)
  - wxT: layer-0 input weights applied to the raw x block (zero for stages 1,2)
  - me/mo masks select which collective's rx feeds the projection
  - maskT zeroes the hidden-state carry at the tick where the real sequence
    start reaches this stage
"""

import numpy as np
import ml_dtypes

S, B, IN, H, NL = 512, 64, 128, 512, 3
NCORES = 8
NWF = 2
BLW = B // NWF            # 32 batch rows per wavefront
JT = H // 128             # 4
KC = H // 128             # 4
FBW = JT * BLW            # 128 folded cols per step
TB = 16                   # timesteps per block
NT = S // TB              # 32 blocks
D = 2                     # ticks per pipeline hop
NTICKS = NT + (NL - 1) * D

CC_GROUPS = [[0, 1, 2], [4, 5, 6], [3, 7]]   # slot0=stage0, slot1=stage1 senders

# core -> (wavefront, stage); stage -1 = spare
ROLE = {0: (0, 0), 1: (0, 1), 2: (0, 2), 3: (0, -1),
        4: (1, 0), 5: (1, 1), 6: (1, 2), 7: (1, -1)}
OUT_CORES = {0: 2, 1: 6}   # wavefront -> core holding top-layer output

_NC_CACHE = {}


def _pack_wT(w):
    jt = w.shape[0] // 128
    kcn = w.shape[1] // 128
    w4 = np.asarray(w, np.float32).reshape(jt, 128, kcn, 128)
    return np.ascontiguousarray(
        w4.transpose(3, 2, 0, 1).reshape(128, kcn * jt * 128)
    ).astype(ml_dtypes.bfloat16)


def _build(s=S, tb=TB):
    from contextlib import ExitStack
    from concourse import bacc, mybir
    import concourse.tile as tile

    f32, bf16 = mybir.dt.float32, mybir.dt.bfloat16
    Tanh = mybir.ActivationFunctionType.Tanh
    NT = s // tb
    NTICKS = NT + (NL - 1) * D
    BK = tb * BLW             # gemm N per j-tile
    FBK = tb * FBW            # cols per block

    nc = bacc.Bacc("TRN2", target_bir_lowering=False, debug=False, num_devices=NCORES)

    xT_d = nc.dram_tensor("xT", [IN, s * BLW], bf16, kind="ExternalInput")
    wxT_d = nc.dram_tensor("wxT", [IN, JT * 128], bf16, kind="ExternalInput")
    wprojT_d = nc.dram_tensor("wprojT", [128, KC * JT * 128], bf16, kind="ExternalInput")
    whhT_d = nc.dram_tensor("whhT", [128, KC * JT * 128], bf16, kind="ExternalInput")
    bias_d = nc.dram_tensor("biasT", [128, JT], f32, kind="ExternalInput")
    mask_d = nc.dram_tensor("maskT", [128, NTICKS + 2], f32, kind="ExternalInput")
    ident_d = nc.dram_tensor("identT", [128, 128], bf16, kind="ExternalInput")
    y_d = nc.dram_tensor("y_out", [128, (NT + 1) * FBK], bf16, kind="ExternalOutput")

    with tile.TileContext(nc) as tc, ExitStack() as ctx:
        consts = ctx.enter_context(tc.tile_pool(name="consts", bufs=1))
        xwp = ctx.enter_context(tc.tile_pool(name="xwp", bufs=2))
        ybp = ctx.enter_context(tc.tile_pool(name="ybp", bufs=2))
        rxp = ctx.enter_context(tc.tile_pool(name="rxp", bufs=2))
        gpsum = ctx.enter_context(tc.tile_pool(name="gpsum", bufs=4, space="PSUM"))
        rpsum = ctx.enter_context(tc.tile_pool(name="rpsum", bufs=4, space="PSUM"))
        small = ctx.enter_context(tc.tile_pool(name="small", bufs=6))
        dram = ctx.enter_context(tc.tile_pool(name="dram", bufs=1, space="DRAM"))

        xT_s = consts.tile([IN, s * BLW], bf16)
        nc.sync.dma_start(xT_s[:], xT_d.ap())
        wxT_s = consts.tile([IN, JT * 128], bf16)
        nc.sync.dma_start(wxT_s[:], wxT_d.ap())
        wprojT_s = consts.tile([128, KC * JT * 128], bf16)
        nc.sync.dma_start(wprojT_s[:], wprojT_d.ap())
        whhT_s = consts.tile([128, KC * JT * 128], bf16)
        nc.sync.dma_start(whhT_s[:], whhT_d.ap())
        bias_s = consts.tile([128, JT], f32)
        nc.sync.dma_start(bias_s[:], bias_d.ap())
        mask_s = consts.tile([128, NTICKS + 2], f32)
        nc.sync.dma_start(mask_s[:], mask_d.ap())
        ident_s = consts.tile([128, 128], bf16)
        nc.sync.dma_start(ident_s[:], ident_d.ap())
        zeros_s = consts.tile([128, FBK], bf16)
        nc.vector.memset(zeros_s[:], 0.0)
        hcar = consts.tile([128, FBW], bf16)
        nc.vector.memset(hcar[:], 0.0)

        # collective buffers (double-buffered by tick parity)
        tx = [dram.tile([128, FBK], bf16, tag=f"tx{i}", name=f"tx{i}") for i in range(2)]
        rx3 = [dram.tile([3 * 128, FBK], bf16, tag=f"rx{i}", name=f"rx{i}") for i in range(2)]
        for rr in rx3:
            for r0 in range(3):
                nc.sync.dma_start(rr[r0 * 128:(r0 + 1) * 128, :], zeros_s[:])
        for tt in tx:
            nc.sync.dma_start(tt[:], zeros_s[:])

        me = mask_s[:, NTICKS:NTICKS + 1]
        mo = mask_s[:, NTICKS + 1:NTICKS + 2]

        for k in range(NTICKS):
            sl = k % 2
            # reset h carry where the real sequence start reaches this stage
            nc.vector.tensor_scalar_mul(hcar[:], hcar[:], mask_s[:, k:k + 1])

            # fetch received block (written by the collectives of tick k-2)
            rxs_e = rxp.tile([128, FBK], bf16, tag="rxse")
            nc.sync.dma_start(rxs_e[:], rx3[sl][0:128, :])
            rxs_o = rxp.tile([128, FBK], bf16, tag="rxso")
            nc.sync.dma_start(rxs_o[:], rx3[sl][128:256, :])
            gin = rxp.tile([128, FBK], bf16, tag="gin")
            nc.vector.tensor_scalar_mul(gin[:], rxs_e[:], me)
            tmp_o = rxp.tile([128, FBK], bf16, tag="tmpo")
            nc.vector.tensor_scalar_mul(tmp_o[:], rxs_o[:], mo)
            nc.vector.tensor_add(gin[:], gin[:], tmp_o[:])
            gin_r = gin[:].rearrange("p (t f) -> p t f", f=FBW)

            # input projection for this block:
            #   xw[t, b, j*128+p] = gin @ wproj^T + x_blk @ wx^T + bias
            xw = xwp.tile([128, FBK], bf16, tag="xw")
            xw_r = xw[:].rearrange("p (t f) -> p t f", f=FBW)
            xoff = min(k, NT - 1) * BK
            for j in range(JT):
                ps = gpsum.tile([128, BK], f32)
                ps3 = ps[:].rearrange("p (t b) -> p t b", b=BLW)
                for kc in range(KC):
                    nc.tensor.matmul(
                        ps3,
                        wprojT_s[:, (kc * JT + j) * 128:(kc * JT + j + 1) * 128],
                        gin_r[:, :, kc * BLW:(kc + 1) * BLW],
                        start=(kc == 0), stop=False,
                    )
                nc.tensor.matmul(
                    ps[:],
                    wxT_s[:, j * 128:(j + 1) * 128],
                    xT_s[:, xoff:xoff + BK],
                    start=False, stop=True,
                )
                nc.vector.tensor_scalar_add(
                    out=xw_r[:, :, j * BLW:(j + 1) * BLW],
                    in0=ps3,
                    scalar1=bias_s[:, j:j + 1],
                )

            # recurrence over the block
            ytb = ybp.tile([128, FBK], bf16, tag="ytb")
            yt_r = ytb[:].rearrange("p (t f) -> p t f", f=FBW)
            prev = hcar[:]
            for t in range(tb):
                ps = rpsum.tile([128, FBW], f32)
                for j in range(JT):
                    for i in range(KC):
                        kc = (j + i) % KC
                        nc.tensor.matmul(
                            ps[:, j * BLW:(j + 1) * BLW],
                            whhT_s[:, (kc * JT + j) * 128:(kc * JT + j + 1) * 128],
                            prev[:, kc * BLW:(kc + 1) * BLW],
                            start=(i == 0), stop=False,
                        )
                    # accumulate xw_t into the same psum block: += I.T @ xw_slab
                    nc.tensor.matmul(
                        ps[:, j * BLW:(j + 1) * BLW],
                        ident_s[:],
                        xw_r[:, t, j * BLW:(j + 1) * BLW],
                        start=False, stop=True,
                    )
                    if j % 2 == 1:
                        nc.scalar.activation(
                            yt_r[:, t, (j - 1) * BLW:(j + 1) * BLW],
                            ps[:, (j - 1) * BLW:(j + 1) * BLW], Tanh)
                prev = yt_r[:, t, :]
            nc.vector.tensor_copy(hcar[:], yt_r[:, tb - 1, :])

            # emit outputs: local DRAM copy + send to successor
            yoff = k - (NL - 1) * D if k >= (NL - 1) * D else NT
            nc.sync.dma_start(y_d.ap()[:, yoff * FBK:(yoff + 1) * FBK], ytb[:])
            nc.sync.dma_start(tx[sl][:], ytb[:])
            nc.gpsimd.collective_compute(
                "AllGather", mybir.AluOpType.bypass,
                replica_groups=CC_GROUPS,
                ins=[tx[sl][:].opt()], outs=[rx3[sl][:].opt()],
            )

    nc.compile()
    return nc


def _in_maps(input_x, w_ih_first, w_ih_rest, w_hh, b_ih, b_hh, s=S, tb=TB):
    bf = ml_dtypes.bfloat16
    NT = s // tb
    NTICKS = NT + (NL - 1) * D
    x = np.asarray(input_x, np.float32)
    zproj = np.zeros((128, KC * JT * 128), bf)
    wx_real = _pack_wT(np.asarray(w_ih_first))
    zx = np.zeros_like(wx_real)
    wproj = [None, _pack_wT(np.asarray(w_ih_rest)[0]), _pack_wT(np.asarray(w_ih_rest)[1])]
    whh = [_pack_wT(np.asarray(w_hh)[l]) for l in range(NL)]
    bsum = np.asarray(b_ih, np.float32) + np.asarray(b_hh, np.float32)

    maps = []
    for c in range(NCORES):
        wf, st = ROLE[c]
        xs = x[:, wf * BLW:(wf + 1) * BLW, :]
        xT = np.ascontiguousarray(xs.transpose(2, 0, 1).reshape(IN, s * BLW)).astype(bf)
        mask = np.ones((128, NTICKS + 2), np.float32)
        if st >= 0:
            mask[:, st * D] = 0.0
        mask[:, NTICKS] = 1.0 if st == 1 else 0.0
        mask[:, NTICKS + 1] = 1.0 if st == 2 else 0.0
        stl = 0 if st < 0 else st
        bias = np.ascontiguousarray(
            bsum[stl].reshape(JT, 128).T.astype(np.float32)) if st >= 0 else np.zeros((128, JT), np.float32)
        maps.append({
            "xT": xT,
            "wxT": wx_real if st == 0 else zx,
            "wprojT": wproj[st] if st in (1, 2) else zproj,
            "whhT": whh[stl] if st >= 0 else np.zeros_like(whh[0]),
            "biasT": bias,
            "maskT": mask,
            "identT": np.eye(128, dtype=np.float32).astype(bf),
        })
    return maps


def _unfold_y(arr, s=S, tb=TB):
    """[128, (nt+1)*tb*FBW] -> [s, BLW, H]."""
    a = np.asarray(arr, np.float32)[:, :s * FBW]
    a = a.reshape(128, s, JT, BLW)
    return np.ascontiguousarray(a.transpose(1, 3, 2, 0).reshape(s, BLW, H))


def kernel(input_x, w_ih_first, w_ih_rest, w_hh, b_ih, b_hh):
    from concourse.bass_utils import run_bass_kernel_spmd

    if "nc" not in _NC_CACHE:
        _NC_CACHE["nc"] = _build()
    nc = _NC_CACHE["nc"]

    maps = _in_maps(input_x, w_ih_first, w_ih_rest, w_hh, b_ih, b_hh)
    res = run_bass_kernel_spmd(nc, maps, core_ids=list(range(NCORES)))
    halves = [_unfold_y(res.results[OUT_CORES[wf]]["y_out"]) for wf in range(NWF)]
    return np.concatenate(halves, axis=1).astype(np.float32)


# revision 19
# speedup vs baseline: 2.6125x; 1.1550x over previous
"""Trainium2 Bass kernel for a 3-layer tanh RNN (SEQ=512, BATCH=64, IN=128, HID=512).

Strategy (v1): data-parallel over batch — 8 NeuronCores, 8 batch rows each,
weights replicated. All compute on device:
  - per-layer input projection xw = y_prev @ W_ih^T + b   (big GEMMs, folded-T layout)
  - sequential 512-step recurrence h_t = tanh(xw_t + h_{t-1} @ W_hh^T)
Layouts keep h transposed+folded ([128 partitions, 4*BL cols] where row p of
j-block is hidden unit j*128+p) so no transposes are needed inside the scan.
Weights/activations bf16 (fp32 PSUM accumulation), xw and output fp32.
"""

import numpy as np
import ml_dtypes

S, B, IN, H, NL = 512, 64, 128, 512, 3
NCORES = 8
BL = B // NCORES          # batch rows per core
JT = H // 128             # output j-tiles
KC = H // 128             # contraction chunks
FB = JT * BL              # folded slab width per timestep

_NC_CACHE = {}


# ---------------- host-side packing ----------------

def _pack_wT(w):
    """[Hout, Hin] -> [128, (Hin/128)*(Hout/128)*128] bf16; tile (kc, j) at
    cols ((kc*JT + j)*128 : +128), lhsT[k=p, m] = w[j*128+m, kc*128+p]."""
    jt = w.shape[0] // 128
    kcn = w.shape[1] // 128
    w4 = np.asarray(w, np.float32).reshape(jt, 128, kcn, 128)
    return np.ascontiguousarray(
        w4.transpose(3, 2, 0, 1).reshape(128, kcn * jt * 128)
    ).astype(ml_dtypes.bfloat16)


def _pack_bias(b_ih, b_hh):
    """[NL, H] x2 -> [128, NL*JT] fp32, col l*JT+j row p = bias[l, j*128+p]."""
    s = np.asarray(b_ih, np.float32) + np.asarray(b_hh, np.float32)
    return np.ascontiguousarray(s.reshape(NL, JT, 128).transpose(2, 0, 1).reshape(128, NL * JT))


def _pack_xT(x_slice):
    """[S, BL, IN] -> [IN, S*BL] bf16 (t-major, b-minor columns)."""
    return np.ascontiguousarray(
        np.asarray(x_slice, np.float32).transpose(2, 0, 1).reshape(IN, -1)
    ).astype(ml_dtypes.bfloat16)


# ---------------- device program ----------------

def _build(nsteps=S, bl=BL, tb=32):
    from contextlib import ExitStack
    from concourse import bacc, mybir
    import concourse.tile as tile

    f32, bf16 = mybir.dt.float32, mybir.dt.bfloat16
    Tanh = mybir.ActivationFunctionType.Tanh
    fb = JT * bl
    tb = min(tb, nsteps)
    NT = nsteps // tb
    TBLK = min(512 // bl, nsteps)
    NBLK = nsteps // TBLK

    nc = bacc.Bacc("TRN2", target_bir_lowering=False, debug=False, num_devices=NCORES)

    xT_d = nc.dram_tensor("xT", [IN, nsteps * bl], bf16, kind="ExternalInput")
    wih0T_d = nc.dram_tensor("wih0T", [IN, JT * 128], bf16, kind="ExternalInput")
    wihT_d = nc.dram_tensor("wihT", [128, (NL - 1) * KC * JT * 128], bf16, kind="ExternalInput")
    whhT_d = nc.dram_tensor("whhT", [128, NL * KC * JT * 128], bf16, kind="ExternalInput")
    bias_d = nc.dram_tensor("biasT", [128, NL * JT], f32, kind="ExternalInput")
    ident_d = nc.dram_tensor("identT", [128, 128], bf16, kind="ExternalInput")
    y_d = nc.dram_tensor("y_out", [128, nsteps * fb], f32, kind="ExternalOutput")

    with tile.TileContext(nc) as tc, ExitStack() as ctx:
        consts = ctx.enter_context(tc.tile_pool(name="consts", bufs=1))
        big = ctx.enter_context(tc.tile_pool(name="big", bufs=1))
        ybp = ctx.enter_context(tc.tile_pool(name="ybp", bufs=2))
        xwp = ctx.enter_context(tc.tile_pool(name="xwp", bufs=2))
        gpsum = ctx.enter_context(tc.tile_pool(name="gpsum", bufs=2, space="PSUM"))
        rpsum = ctx.enter_context(tc.tile_pool(name="rpsum", bufs=6, space="PSUM"))
        ring = ctx.enter_context(tc.tile_pool(name="ring", bufs=4))
        ysp = ctx.enter_context(tc.tile_pool(name="ystage", bufs=2))

        xT_s = consts.tile([IN, nsteps * bl], bf16)
        nc.sync.dma_start(xT_s[:], xT_d.ap())
        wih0T_s = consts.tile([IN, JT * 128], bf16)
        nc.sync.dma_start(wih0T_s[:], wih0T_d.ap())
        wihT_s = consts.tile([128, (NL - 1) * KC * JT * 128], bf16)
        nc.sync.dma_start(wihT_s[:], wihT_d.ap())
        whhT_s = consts.tile([128, NL * KC * JT * 128], bf16)
        nc.sync.dma_start(whhT_s[:], whhT_d.ap())
        bias_s = consts.tile([128, NL * JT], f32)
        nc.sync.dma_start(bias_s[:], bias_d.ap())
        ident_s = consts.tile([128, 128], bf16)
        nc.sync.dma_start(ident_s[:], ident_d.ap())
        zeros_s = consts.tile([128, fb], bf16)
        nc.vector.memset(zeros_s[:], 0.0)

        # layer-0 projection for all timesteps upfront
        xw0 = big.tile([128, nsteps * fb], bf16)
        xw0_r = xw0[:].rearrange("p (t f) -> p t f", f=fb)
        for j in range(JT):
            for nb in range(NBLK):
                ps = gpsum.tile([128, TBLK * bl], f32, tag="gp", name=f"g0_{j}_{nb}")
                nc.tensor.matmul(ps[:], wih0T_s[:, j * 128:(j + 1) * 128],
                                 xT_s[:, nb * TBLK * bl:(nb + 1) * TBLK * bl],
                                 start=True, stop=True)
                nc.vector.tensor_scalar_add(
                    out=xw0_r[:, nb * TBLK:(nb + 1) * TBLK, j * bl:(j + 1) * bl],
                    in0=ps[:].rearrange("p (t b) -> p t b", b=bl),
                    scalar1=bias_s[:, j:j + 1])

        ybuf = {0: {}, 1: {}}
        xwbuf = {1: {}, 2: {}}
        ystage = {}
        lasth = [zeros_s[:]]

        def emit_gemm(l, bb):
            src_r = ybuf[l - 1][bb][:].rearrange("p (t f) -> p t f", f=fb)
            xw = xwp.tile([128, tb * fb], bf16, tag=f"xw{l}", name=f"xw{l}_{bb}")
            xw_r = xw[:].rearrange("p (t f) -> p t f", f=fb)
            for j in range(JT):
                ps = gpsum.tile([128, tb * bl], f32, tag="gp", name=f"g{l}_{bb}_{j}")
                ps3 = ps[:].rearrange("p (t b) -> p t b", b=bl)
                for kc in range(KC):
                    base = ((l - 1) * KC + kc) * JT + j
                    nc.tensor.matmul(ps3, wihT_s[:, base * 128:(base + 1) * 128],
                                     src_r[:, :, kc * bl:(kc + 1) * bl],
                                     start=(kc == 0), stop=(kc == KC - 1))
                nc.vector.tensor_scalar_add(
                    out=xw_r[:, :, j * bl:(j + 1) * bl], in0=ps3,
                    scalar1=bias_s[:, l * JT + j:l * JT + j + 1])
            xwbuf[l][bb] = xw

        def rec_step(l, bb, i):
            top = l == NL - 1
            if i == 0 and not top:
                ybuf[l][bb] = ybp.tile([128, tb * fb], bf16, tag=f"y{l}", name=f"y{l}_{bb}")
            if i == 0 and top:
                ystage[bb] = ysp.tile([128, tb * fb], f32, tag="ys", name=f"ys_{bb}")
            if top:
                prev = lasth[0]
            elif i == 0:
                prev = zeros_s[:] if bb == 0 else ybuf[l][bb - 1][:, (tb - 1) * fb:tb * fb]
            else:
                prev = ybuf[l][bb][:, (i - 1) * fb:i * fb]
            xw_r = xw0_r[:, bb * tb + i, :] if l == 0 else \
                xwbuf[l][bb][:].rearrange("p (t f) -> p t f", f=fb)[:, i, :]
            ps = rpsum.tile([128, fb], f32, tag="rp", name=f"r{l}_{bb}_{i}")
            # whole-slab identity matmul FIRST: initializes psum with xw_t in one
            # op and is independent of h_{t-1}, so it runs ahead of the chain
            nc.tensor.matmul(ps[:], ident_s[:], xw_r,
                             start=True, stop=False, skip_group_check=True)
            nmm = 0
            for j in range(JT):
                for ii in range(KC):
                    kc = (j + ii) % KC
                    base = (l * KC + kc) * JT + j
                    nmm += 1
                    nc.tensor.matmul(ps[:, j * bl:(j + 1) * bl],
                                     whhT_s[:, base * 128:(base + 1) * 128],
                                     prev[:, kc * bl:(kc + 1) * bl],
                                     start=False, stop=(nmm == JT * KC),
                                     skip_group_check=True)
            if not top:
                nc.scalar.activation(ybuf[l][bb][:, i * fb:(i + 1) * fb], ps[:], Tanh)
            else:
                out_slab = ystage[bb][:, i * fb:(i + 1) * fb]
                nc.scalar.activation(out_slab, ps[:], Tanh)
                hT = ring.tile([128, fb], bf16, tag="hT", name=f"h_{bb}_{i}")
                nc.vector.tensor_copy(hT[:], out_slab)
                lasth[0] = hT[:]
                if i == tb - 1:
                    c0 = bb * tb * fb
                    nc.sync.dma_start(y_d.ap()[:, c0:c0 + tb * fb], ystage[bb][:])

        for b in range(NT + NL - 1):
            if 1 <= b <= NT:
                emit_gemm(1, b - 1)
            if 2 <= b <= NT + 1:
                emit_gemm(2, b - 2)
            for i in range(tb):
                if b < NT:
                    rec_step(0, b, i)
                if 0 <= b - 1 < NT:
                    rec_step(1, b - 1, i)
                if 0 <= b - 2 < NT:
                    rec_step(2, b - 2, i)

    nc.compile()
    return nc


def _unfold_y(arr, nsteps=S, bl=BL):
    """[p, t*fb + j*bl + b] fp32 -> [t, b, j*128+p]."""
    return np.ascontiguousarray(
        arr.reshape(128, nsteps, JT, bl).transpose(1, 3, 2, 0).reshape(nsteps, bl, H)
    )


def _in_maps(input_x, w_ih_first, w_ih_rest, w_hh, b_ih, b_hh, nsteps=S, bl=BL, ncores=NCORES):
    wih0T = _pack_wT(np.asarray(w_ih_first))
    wihT = np.concatenate([_pack_wT(np.asarray(w_ih_rest)[i]) for i in range(NL - 1)], axis=1)
    whhT = np.concatenate([_pack_wT(np.asarray(w_hh)[i]) for i in range(NL)], axis=1)
    biasT = _pack_bias(b_ih, b_hh)
    maps = []
    for c in range(ncores):
        xs = np.asarray(input_x)[:nsteps, c * bl:(c + 1) * bl, :]
        maps.append({
            "xT": _pack_xT(xs),
            "wih0T": wih0T,
            "wihT": wihT,
            "whhT": whhT,
            "biasT": biasT,
            "identT": np.eye(128, dtype=np.float32).astype(ml_dtypes.bfloat16),
        })
    return maps


def kernel(input_x, w_ih_first, w_ih_rest, w_hh, b_ih, b_hh):
    from concourse.bass_utils import run_bass_kernel_spmd

    key = (S, BL)
    if key not in _NC_CACHE:
        _NC_CACHE[key] = _build(S, BL)
    nc = _NC_CACHE[key]

    maps = _in_maps(input_x, w_ih_first, w_ih_rest, w_hh, b_ih, b_hh)
    res = run_bass_kernel_spmd(nc, maps, core_ids=list(range(NCORES)))
    outs = [_unfold_y(res.results[c]["y_out"]) for c in range(NCORES)]
    return np.concatenate(outs, axis=1).astype(np.float32)
